# revision 4
# baseline (speedup 1.0000x reference)
"""Trainium2 Bass kernel for single-head causal attention.

Problem: x[4, 2048, 1024] fp32; wq/wk/wv [1024, 1024] (torch layout [d_out, d_in]).
  q = x @ wq.T ; k = x @ wk.T ; v = x @ wv.T  (per batch)
  out = softmax(causal(q @ k.T) / 32) @ v

Sharding (8 cores): core c = 2*b + h owns batch b and half of its query rows.
Query rows are split between the two cores of a batch by parity *within* each
1024-row group so both cores see an identical causal work profile -> the SPMD
program is fully uniform; only data (inputs) differ per core.

All matmuls are bf16 (1 cycle/row PE rate, cheap LDWEIGHTS, half the DMA
bytes of fp32); PSUM accumulation stays fp32.

Per-core device program:
  phase 1a: kT[o, s] = wkT-chunks.T @ xkvT  (k transposed layout, SBUF-resident)
  phase 1b: v[s, o]  = xkvT-chunks.T @ wvT  -> spilled to DRAM scratch
  phase 1c: qT[o, s_own] = wqT-chunks.T @ xqT  (SBUF-resident)
  phase 2:  per group g (2 groups of 512 own-q columns, kb = 8g+8 key blocks):
      scoresT[k, q'] = sum_o kT.T @ qT  (PSUM, 8 o-chunk matmuls, N=512)
      additive causal mask (DVE, bf16 -1e9 tiles) on the last 8 key blocks;
      p = exp(scores/32) (ACT, PSUM->SBUF fp32r); l[q'] += ones.T @ p (PE);
      AV: out[q',o] += p.T-slices @ v-blocks; final ACT copy divides by l via
      per-partition scale AP (l transposed into lanes by 4 tiny column DMAs).
"""

import os
import sys
import types
from contextlib import ExitStack

for _p in ("/opt/trn_rl_repo", "/root/.axon_site/_ro/trn_rl_repo"):
    if os.path.isdir(_p) and _p not in sys.path:
        sys.path.insert(0, _p)

import numpy as np

import concourse.bacc as bacc
import concourse.mybir as mybir
import concourse.tile as tile
from concourse.bass_utils import run_bass_kernel_spmd

F32 = mybir.dt.float32
F32R = mybir.dt.float32r
BF16 = mybir.dt.bfloat16

B, S, D = 4, 2048, 1024
P = 128
DC = D // P      # 8 contraction chunks
OC = D // P      # 8 output-dim chunks
SKB = S // P     # 16 key blocks
G = 2            # query groups per core
QW = 512         # query columns per group per core
SQ = G * QW      # 1024 own query rows per core
N_CORES = 8
SCALE = 1.0 / 32.0  # 1/sqrt(D)
NEG = -1e9


def _install_axon_profile_hook():
    """Provide antenv.axon_hooks (absent in this image) so trace=True works."""
    name = "antenv.axon_hooks"
    if name in sys.modules:
        return
    mod = types.ModuleType(name)
    _hook = [None]
    mod.set_axon_ntff_profile_hook = lambda h: _hook.__setitem__(0, h)
    mod.get_axon_ntff_profile_hook = lambda: _hook[0]
    sys.modules[name] = mod
    try:
        import antenv

        antenv.axon_hooks = mod
        from trn_agent_boot.trn_boot import _ntff_profile_via_ctypes

        mod.set_axon_ntff_profile_hook(
            _ntff_profile_via_ctypes("/opt/axon/libaxon_pjrt.so")
        )
    except Exception:
        pass


def _round_fp32r(a):
    """Round fp32 to fp32r (11 stored mantissa bits, RNE) as the PE expects."""
    u = np.ascontiguousarray(a, dtype=np.float32).view(np.uint32)
    r = (u + np.uint32(0x7FF) + ((u >> np.uint32(12)) & np.uint32(1))) \
        & np.uint32(0xFFFFF000)
    return r.view(np.float32)


def _build_program():
    nc = bacc.Bacc("TRN2", target_bir_lowering=False, debug=False,
                   num_devices=N_CORES)

    xkv = nc.dram_tensor("xkv", [D, S], BF16, kind="ExternalInput").ap()
    xq = nc.dram_tensor("xq", [D, SQ], BF16, kind="ExternalInput").ap()
    wqt = nc.dram_tensor("wqt", [D, D], BF16, kind="ExternalInput").ap()
    wkt = nc.dram_tensor("wkt", [D, D], BF16, kind="ExternalInput").ap()
    wvt = nc.dram_tensor("wvt", [D, D], BF16, kind="ExternalInput").ap()
    ones_in = nc.dram_tensor("ones", [P, 1], BF16, kind="ExternalInput").ap()
    mask = nc.dram_tensor("mask", [P, 8 * QW], BF16, kind="ExternalInput").ap()
    out = nc.dram_tensor("out", [SQ, D], F32, kind="ExternalOutput").ap()
    vdram = nc.dram_tensor("vscratch", [S, D], BF16).ap()

    NQ = S // 512  # 4 s-quarters of xkv

    with tile.TileContext(nc, pool_alloc_mode="queue") as tc, ExitStack() as es:
        const = es.enter_context(tc.tile_pool(name="const", bufs=1))
        ktpool = es.enter_context(tc.tile_pool(name="ktpool", bufs=8))
        qtpool = es.enter_context(tc.tile_pool(name="qtpool", bufs=8))

        ones1 = const.tile([P, 1], BF16)
        nc.sync.dma_start(out=ones1[:], in_=ones_in[:])

        p1 = ExitStack()
        # weight o-quarter tiles [128, 8*256]: quarter i holds o-cols
        # [256i, 256(i+1)) for all 8 d-chunks, one 1 MiB DMA each.
        # One shared pool cycles wk -> wv -> wq.
        wpool = p1.enter_context(tc.tile_pool(name="wpool", bufs=5))
        # x s-tiles [128, 8*512]: tile holds 512 s-cols for all 8 d-chunks,
        # one 2 MiB DMA each; shared pool cycles xkv -> xq
        xpool = p1.enter_context(tc.tile_pool(name="xpool", bufs=4))
        vstage = p1.enter_context(tc.tile_pool(name="vstage", bufs=3))
        pp = p1.enter_context(tc.tile_pool(name="pp", bufs=2, space="PSUM"))


        def load_w(src, i):
            # [p, dc*256 + f] = src[dc*128 + p, i*256 + f]
            w = wpool.tile([P, DC * 256], BF16, tag="w")
            nc.sync.dma_start(
                out=w.rearrange("p (c f) -> p c f", c=DC),
                in_=src[:, i * 256:(i + 1) * 256].rearrange(
                    "(c p) f -> p c f", p=P))
            return w

        def load_x(src, st, eng=None):
            # [p, dc*512 + f] = src[dc*128 + p, st*512 + f]
            t = xpool.tile([P, DC * 512], BF16, tag="x")
            (eng or nc.sync).dma_start(
                out=t.rearrange("p (c f) -> p c f", c=DC),
                in_=src[:, st * 512:(st + 1) * 512].rearrange(
                    "(c p) f -> p c f", p=P))
            return t

        def w_lhsT(w, oc, dc):
            # [128(d), 128(o)] slice for output chunk oc, contraction chunk dc
            base = dc * 256 + (oc % 2) * P
            return w[oc // 2][:, base:base + P]

        # startup order: first wk quarter + first xkv s-tile unblock the
        # first accumulation group as early as possible
        wk_sb = [load_w(wkt, 0)]
        xkv_sb = [load_x(xkv, 0)]

        wk_sb += [load_w(wkt, i) for i in range(1, 4)]
        xkv_sb += [load_x(xkv, st) for st in range(1, NQ)]


        # ---- phase 1a: kT projection (resident) ----
        # st-outer: the first xkv quarter serves 8 accumulation groups while
        # the next quarter streams in
        kt = [ktpool.tile([P, S], BF16, tag="kt", name=f"kt{i}") for i in range(OC)]
        for st in range(NQ):
            for oc in range(OC):
                ps = pp.tile([P, 512], F32, tag="pp")
                for dc in range(DC):
                    nc.tensor.matmul(
                        ps[:],
                        w_lhsT(wk_sb, oc, dc),
                        xkv_sb[st][:, dc * 512:(dc + 1) * 512],
                        start=(dc == 0), stop=(dc == DC - 1),
                    )
                nc.vector.tensor_copy(
                    kt[oc][:, st * 512:(st + 1) * 512], ps[:])

        # ---- phase 1b: v projection -> DRAM spill (o-quarters, N=256) ----
        wv_sb = [load_w(wvt, i) for i in range(4)]
        for oh in range(2):
            for sb in range(SKB):
                st, r = divmod(sb, 4)
                vt = vstage.tile([P, 512], BF16, tag="v")
                for oq in (2 * oh, 2 * oh + 1):
                    ps = pp.tile([P, 256], F32, tag="pp")
                    for dc in range(DC):
                        nc.tensor.matmul(
                            ps[:],
                            xkv_sb[st][:, dc * 512 + r * P:dc * 512 + (r + 1) * P],
                            wv_sb[oq][:, dc * 256:(dc + 1) * 256],
                            start=(dc == 0), stop=(dc == DC - 1),
                        )
                    nc.vector.tensor_copy(
                        vt[:, (oq % 2) * 256:(oq % 2 + 1) * 256], ps[:])
                nc.scalar.dma_start(
                    out=vdram[sb * P:(sb + 1) * P, oh * 512:(oh + 1) * 512],
                    in_=vt[:])

        # ---- phase 1c: qT projection (resident) ----
        wq_sb = [load_w(wqt, i) for i in range(4)]
        xq_sb = [load_x(xq, st, eng=nc.gpsimd) for st in range(2)]
        qt = [qtpool.tile([P, SQ], BF16, tag="qt", name=f"qt{i}")
              for i in range(OC)]
        for st in range(SQ // 512):
            for oc in range(OC):
                ps = pp.tile([P, 512], F32, tag="pp")
                for dc in range(DC):
                    nc.tensor.matmul(
                        ps[:],
                        w_lhsT(wq_sb, oc, dc),
                        xq_sb[st][:, dc * 512:(dc + 1) * 512],
                        start=(dc == 0), stop=(dc == DC - 1),
                    )
                nc.vector.tensor_copy(
                    qt[oc][:, st * 512:(st + 1) * 512], ps[:])
        p1.close()

        # ---- phase 2: attention ----
        vpool = es.enter_context(tc.tile_pool(name="vpool", bufs=16))
        maskpool = es.enter_context(tc.tile_pool(name="maskpool", bufs=1))
        ptpool = es.enter_context(tc.tile_pool(name="ptpool", bufs=16))
        linvpool = es.enter_context(tc.tile_pool(name="linvpool", bufs=1))
        linvtpool = es.enter_context(tc.tile_pool(name="linvtpool", bufs=2))
        outpool = es.enter_context(tc.tile_pool(name="outpool", bufs=2))
        ps_s = es.enter_context(tc.tile_pool(name="ps_s", bufs=2, space="PSUM"))
        ps_l = es.enter_context(tc.tile_pool(name="ps_l", bufs=2, space="PSUM"))
        ps_av = es.enter_context(tc.tile_pool(name="ps_av", bufs=4, space="PSUM"))

        # the causal diagonal-band mask is identical for both groups: load once
        # first (exp of every g0 unit needs it), then v blocks split across
        # both HWDGE queues so the first AV pass isn't starved
        mask_sb = maskpool.tile([P, 8 * QW], BF16, tag="mask")
        nc.sync.dma_start(out=mask_sb[:], in_=mask[:])
        v_sb = []
        for kb in range(SKB):
            t = vpool.tile([P, D], BF16, tag="vres")
            nc.sync.dma_start(out=t[:], in_=vdram[kb * P:(kb + 1) * P, :])
            v_sb.append(t)

        # Per-group state; group g has U = 8g+8 key-block units.
        l_ps = {}
        pts = {0: [None] * 8, 1: [None] * 16}
        score_ps = {0: [None] * 8, 1: [None] * 16}
        linv_t = {}

        def emit_scores(g, j):
            # key block j contributes only to q' >= 64*(j-8g); restrict the
            # scores matmul to that range (clamped to N>=256 for full-rate
            # fp32r; the masked remainder is killed by the additive mask).
            # Units 0/1 of g0 stay full-width so the first use of each ps_s
            # slot initializes the whole bank.
            jj = j - 8 * g
            rs = min(64 * jj, QW - 256) if (j >= 8 * g and j >= 2) else 0
            ps = ps_s.tile([P, QW], F32, tag="s")
            for oc in range(OC):
                nc.tensor.matmul(
                    ps[:, rs:],
                    kt[oc][:, j * P:(j + 1) * P],
                    qt[oc][:, g * QW + rs:(g + 1) * QW],
                    start=(oc == 0), stop=(oc == OC - 1),
                )
            score_ps[g][j] = ps

        def emit_post(g, j):
            # (additive causal mask) + exp + l-accumulation for unit j
            U = 8 * g + 8
            if j >= 8 * g:
                nc.vector.tensor_add(
                    score_ps[g][j][:], score_ps[g][j][:],
                    mask_sb[:, (j - 8 * g) * QW:(j - 8 * g + 1) * QW])
            pt = ptpool.tile([P, QW], BF16, tag="pt")
            nc.scalar.activation(
                pt[:], score_ps[g][j][:],
                mybir.ActivationFunctionType.Exp, scale=SCALE)
            nc.tensor.matmul(
                l_ps[g][:], ones1[:], pt[:],
                start=(j == 0), stop=(j == U - 1),
            )
            pts[g][j] = pt

        def emit_unit_range(g, lo, hi):
            for j in range(lo, hi):
                emit_scores(g, j)
                if j > lo:
                    emit_post(g, j - 1)
            emit_post(g, hi - 1)

        def emit_linv(g):
            linv = linvpool.tile([1, QW], F32, tag="linv")
            nc.vector.reciprocal(linv[:], l_ps[g][:])
            lt = linvtpool.tile([P, 4], F32, tag="linvt")
            for c in range(4):
                nc.scalar.dma_start(
                    out=lt[:, c:c + 1], in_=linv[0:1, c * P:(c + 1) * P])
            linv_t[g] = lt

        def emit_av(g):
            U = 8 * g + 8
            for qs in range(4):
                jmax = min(U, 8 * g + 2 * qs + 2)
                for ot in range(2):
                    ps = ps_av.tile([P, 512], F32, tag="av")
                    for j in range(jmax):
                        nc.tensor.matmul(
                            ps[:],
                            pts[g][j][:, qs * P:(qs + 1) * P],
                            v_sb[j][:, ot * 512:(ot + 1) * 512],
                            start=(j == 0), stop=(j == jmax - 1),
                        )
                    out_sb = outpool.tile([P, 512], F32, tag="out")
                    nc.scalar.mul(out_sb[:], ps[:], linv_t[g][:, qs:qs + 1])
                    r0 = g * 4 * P + qs * P
                    nc.sync.dma_start(
                        out=out[r0:r0 + P, ot * 512:(ot + 1) * 512],
                        in_=out_sb[:])

        l_ps[0] = ps_l.tile([1, QW], F32, tag="l", name="l0")
        l_ps[1] = ps_l.tile([1, QW], F32, tag="l", name="l1")
        emit_unit_range(0, 0, 8)      # g0 scores/exp/l
        emit_linv(0)
        emit_unit_range(1, 0, 8)      # g1 first half: no v dependency, hides
        emit_av(0)                    # the v reload under these scores
        emit_unit_range(1, 8, 16)
        emit_linv(1)
        emit_av(1)

    nc.compile()
    return nc


_PROGRAM = None


def _get_program():
    global _PROGRAM
    if _PROGRAM is None:
        _PROGRAM = _build_program()
    return _PROGRAM


# Set by kernel() after each run: BassKernelResults (exec_time_ns etc.)
last_results = None


def kernel(**inputs):
    global last_results
    _install_axon_profile_hook()

    x = np.asarray(inputs["x"], dtype=np.float32)
    wq = np.asarray(inputs["wq"], dtype=np.float32)
    wk = np.asarray(inputs["wk"], dtype=np.float32)
    wv = np.asarray(inputs["wv"], dtype=np.float32)

    import ml_dtypes
    wqt = np.ascontiguousarray(wq.T).astype(ml_dtypes.bfloat16)
    wkt = np.ascontiguousarray(wk.T).astype(ml_dtypes.bfloat16)
    wvt = np.ascontiguousarray(wv.T).astype(ml_dtypes.bfloat16)

    # own query rows per core half h: parity-h rows within each 1024-row group
    own_rows = {}
    for h in range(2):
        rows = []
        for g in range(G):
            rows.extend(range(1024 * g + h, 1024 * (g + 1), 2))
        own_rows[h] = np.array(rows, dtype=np.int64)

    # additive causal mask tiles [128, 16*512] bf16:
    # tile t (= key block t) masks group t//8's diagonal band
    masks = {}
    kl = np.arange(P)[:, None]
    qp = np.arange(QW)[None, :]
    for h in range(2):
        m = np.zeros((P, 8 * QW), dtype=np.float32)
        for t in range(8):
            krow = P * t + kl
            qrow = 2 * qp + h
            m[:, t * QW:(t + 1) * QW] = np.where(krow <= qrow, 0.0, NEG)
        masks[h] = m.astype(ml_dtypes.bfloat16)

    in_maps = []
    for c in range(N_CORES):
        b, h = divmod(c, 2)
        xt = np.ascontiguousarray(x[b].T).astype(ml_dtypes.bfloat16)  # [D, S]
        in_maps.append({
            "xkv": xt,
            "xq": np.ascontiguousarray(xt[:, own_rows[h]]),
            "wqt": wqt, "wkt": wkt, "wvt": wvt,
            "mask": masks[h],
            "ones": np.ones((P, 1), dtype=ml_dtypes.bfloat16),
        })

    nc = _get_program()
    trace = bool(int(os.environ.get("KERNEL_TRACE", "0")))
    kwargs = {}
    if trace:
        kwargs["trace"] = True
        kwargs["trace_cores"] = list(range(N_CORES))
        tdir = os.environ.get("KERNEL_TRACE_DIR")
        if tdir:
            os.makedirs(tdir, exist_ok=True)
            kwargs["tmpdir"] = tdir
    res = run_bass_kernel_spmd(nc, in_maps, core_ids=list(range(N_CORES)),
                               **kwargs)
    last_results = res

    out = np.empty((B, S, D), dtype=np.float32)
    for c in range(N_CORES):
        b, h = divmod(c, 2)
        out[b, own_rows[h], :] = res.results[c]["out"]
    return out



# revision 8
# speedup vs baseline: 1.0799x; 1.0799x over previous
"""Trainium2 Bass kernel for single-head causal attention.

Problem: x[4, 2048, 1024] fp32; wq/wk/wv [1024, 1024] (torch layout [d_out, d_in]).
  q = x @ wq.T ; k = x @ wk.T ; v = x @ wv.T  (per batch)
  out = softmax(causal(q @ k.T) / 32) @ v

Sharding (8 cores): core c = 2*b + h owns batch b and half of its query rows.
Query rows are split between the two cores of a batch by parity *within* each
1024-row group so both cores see an identical causal work profile -> the SPMD
program is fully uniform; only data (inputs) differ per core.

K/V projections are split by sequence half between the two cores of a batch:
core h projects K/V for s-rows [1024h, 1024h+1024) only, spills the half to
DRAM, and a pair AllGather ([[0,1],[2,3],[4,5],[6,7]]) assembles the full K/V
(in global s order, since rank h's shard is s-half h) which both cores then
reload. This halves the projection FLOPs vs each core projecting full K/V;
the collective overlaps with the V/Q projections.

All matmuls are bf16 (1 cycle/row PE rate at any N, cheap LDWEIGHTS, half the
DMA bytes of fp32); PSUM accumulation stays fp32.

Per-core device program:
  phase 1a: kT_own[o, s_own] = wkT-chunks.T @ xh  -> spill -> AllGather
  phase 1b: v_own[s_own, o]  = xh-chunks.T @ wvT  -> spill -> AllGather
  phase 1c: qT[o, s_own] = wqT-chunks.T @ xqT  (SBUF-resident)
  reload kT[o, 0:2048], v[0:2048, o] from the gathered buffers
  phase 2:  per group g (2 groups of 512 own-q columns, kb = 8g+8 key blocks):
      scoresT[k, q'] = sum_o kT.T @ qT  (PSUM, 8 o-chunk matmuls, exact
      causal width per key block); additive causal mask (DVE, bf16 -1e9
      tiles); p = exp(scores/32) (ACT, PSUM->SBUF bf16); l[q'] += ones.T @ p
      (PE); AV: out[q',o] += p.T-slices @ v-blocks; final ACT copy divides by
      l via per-partition scale AP (l transposed into lanes by 4 tiny column
      DMAs, then reciprocal on [128,4]).
"""

import os
import sys
import types
from contextlib import ExitStack

for _p in ("/opt/trn_rl_repo", "/root/.axon_site/_ro/trn_rl_repo"):
    if os.path.isdir(_p) and _p not in sys.path:
        sys.path.insert(0, _p)

import numpy as np

import concourse.bacc as bacc
import concourse.mybir as mybir
import concourse.tile as tile
from concourse.bass_utils import run_bass_kernel_spmd

F32 = mybir.dt.float32
BF16 = mybir.dt.bfloat16

B, S, D = 4, 2048, 1024
P = 128
DC = D // P      # 8 contraction chunks
OC = D // P      # 8 output-dim chunks
SKB = S // P     # 16 key blocks
SH = S // 2      # 1024 own s-half rows (K/V projection ownership)
G = 2            # query groups per core
QW = 512         # query columns per group per core
SQ = G * QW      # 1024 own query rows per core
N_CORES = 8
PAIRS = [[0, 1], [2, 3], [4, 5], [6, 7]]
SCALE = 1.0 / 32.0  # 1/sqrt(D)
NEG = -1e9


def _install_axon_profile_hook():
    """Provide antenv.axon_hooks (absent in this image) so trace=True works."""
    name = "antenv.axon_hooks"
    if name in sys.modules:
        return
    mod = types.ModuleType(name)
    _hook = [None]
    mod.set_axon_ntff_profile_hook = lambda h: _hook.__setitem__(0, h)
    mod.get_axon_ntff_profile_hook = lambda: _hook[0]
    sys.modules[name] = mod
    try:
        import antenv

        antenv.axon_hooks = mod
        from trn_agent_boot.trn_boot import _ntff_profile_via_ctypes

        mod.set_axon_ntff_profile_hook(
            _ntff_profile_via_ctypes("/opt/axon/libaxon_pjrt.so")
        )
    except Exception:
        pass


def _build_program():
    nc = bacc.Bacc("TRN2", target_bir_lowering=False, debug=False,
                   num_devices=N_CORES)

    xh = nc.dram_tensor("xh", [D, SH], BF16, kind="ExternalInput").ap()
    xq = nc.dram_tensor("xq", [D, SQ], BF16, kind="ExternalInput").ap()
    wqt = nc.dram_tensor("wqt", [D, D], BF16, kind="ExternalInput").ap()
    wkt = nc.dram_tensor("wkt", [D, D], BF16, kind="ExternalInput").ap()
    wvt = nc.dram_tensor("wvt", [D, D], BF16, kind="ExternalInput").ap()
    ones_in = nc.dram_tensor("ones", [P, 1], BF16, kind="ExternalInput").ap()
    mask = nc.dram_tensor("mask", [P, 8 * QW], BF16, kind="ExternalInput").ap()
    out = nc.dram_tensor("out", [SQ, D], F32, kind="ExternalOutput").ap()
    # K/V halves: local spill -> pair AllGather -> full tensors (s order =
    # [rank0 half, rank1 half] = global order).
    kin = nc.dram_tensor("kin", [D, SH], BF16).ap()          # rows oc*128+p
    kout = nc.dram_tensor("kout", [2 * D, SH], BF16).ap()
    vin = nc.dram_tensor("vin", [SH, D], BF16).ap()          # rows sb*128+p
    vout = nc.dram_tensor("vout", [S, D], BF16).ap()

    with tile.TileContext(nc, pool_alloc_mode="queue") as tc, ExitStack() as es:
        const = es.enter_context(tc.tile_pool(name="const", bufs=1))
        ktpool = es.enter_context(tc.tile_pool(name="ktpool", bufs=8))
        qtpool = es.enter_context(tc.tile_pool(name="qtpool", bufs=8))

        ones1 = const.tile([P, 1], BF16)
        nc.gpsimd.dma_start(out=ones1[:], in_=ones_in[:])

        p1 = ExitStack()
        # weight o-quarter tiles [128, 8*256]: quarter i holds o-cols
        # [256i, 256(i+1)) for all 8 d-chunks (for wk/wq: lhsT slices);
        # wv loads as 2 o-half tiles [128, 8*512] (moving operand).
        wpool = p1.enter_context(tc.tile_pool(name="wpool", bufs=5))
        wvpool = p1.enter_context(tc.tile_pool(name="wvpool", bufs=2))
        # x s-tiles [128, 8*512]: tile holds 512 s-cols for all 8 d-chunks
        xpool = p1.enter_context(tc.tile_pool(name="xpool", bufs=4))
        kstage = p1.enter_context(tc.tile_pool(name="kstage", bufs=4))
        vstage = p1.enter_context(tc.tile_pool(name="vstage", bufs=4))
        pp = p1.enter_context(tc.tile_pool(name="pp", bufs=2, space="PSUM"))

        def load_w(src, i, eng=None):
            # [p, dc*256 + f] = src[dc*128 + p, i*256 + f]
            w = wpool.tile([P, DC * 256], BF16, tag="w")
            (eng or nc.sync).dma_start(
                out=w.rearrange("p (c f) -> p c f", c=DC),
                in_=src[:, i * 256:(i + 1) * 256].rearrange(
                    "(c p) f -> p c f", p=P))
            return w

        def load_w_half(src, i, eng=None):
            # [p, dc*512 + f] = src[dc*128 + p, i*512 + f]
            w = wvpool.tile([P, DC * 512], BF16, tag="wv")
            (eng or nc.sync).dma_start(
                out=w.rearrange("p (c f) -> p c f", c=DC),
                in_=src[:, i * 512:(i + 1) * 512].rearrange(
                    "(c p) f -> p c f", p=P))
            return w

        def load_x(src, st, eng=None):
            # [p, dc*512 + f] = src[dc*128 + p, st*512 + f]
            t = xpool.tile([P, DC * 512], BF16, tag="x")
            (eng or nc.sync).dma_start(
                out=t.rearrange("p (c f) -> p c f", c=DC),
                in_=src[:, st * 512:(st + 1) * 512].rearrange(
                    "(c p) f -> p c f", p=P))
            return t

        def w_lhsT(w, oc, dc):
            # [128(d), 128(o)] slice for output chunk oc, contraction chunk dc
            base = dc * 256 + (oc % 2) * P
            return w[oc // 2][:, base:base + P]

        # startup order: first wk quarter + first xh s-tile unblock the
        # first accumulation group as early as possible (separate engines so
        # descriptor generation and queue transfer overlap)
        wk_sb = [load_w(wkt, 0, eng=nc.sync)]
        xh_sb = [load_x(xh, 0, eng=nc.scalar)]
        wk_sb += [load_w(wkt, 1, eng=nc.scalar), load_w(wkt, 2),
                  load_w(wkt, 3)]
        xh_sb.append(load_x(xh, 1))

        # ---- phase 1a: own-half kT projection -> spill to kin ----
        for st in range(2):
            for oc in range(OC):
                ps = pp.tile([P, 512], F32, tag="pp")
                for dc in range(DC):
                    nc.tensor.matmul(
                        ps[:],
                        w_lhsT(wk_sb, oc, dc),
                        xh_sb[st][:, dc * 512:(dc + 1) * 512],
                        start=(dc == 0), stop=(dc == DC - 1),
                    )
                kst = kstage.tile([P, 512], BF16, tag="kst")
                nc.vector.tensor_copy(kst[:], ps[:])
                nc.scalar.dma_start(
                    out=kin[oc * P:(oc + 1) * P, st * 512:(st + 1) * 512],
                    in_=kst[:])

        nc.gpsimd.collective_compute(
            "AllGather", mybir.AluOpType.bypass, replica_groups=PAIRS,
            ins=[kin.opt()], outs=[kout.opt()])

        # ---- phase 1b: own-half v projection -> spill to vin ----
        wv_sb = [load_w_half(wvt, 0), load_w_half(wvt, 1)]
        for sb in range(8):
            st, r = divmod(sb, 4)
            for oh in range(2):
                ps = pp.tile([P, 512], F32, tag="pp")
                for dc in range(DC):
                    nc.tensor.matmul(
                        ps[:],
                        xh_sb[st][:, dc * 512 + r * P:dc * 512 + (r + 1) * P],
                        wv_sb[oh][:, dc * 512:(dc + 1) * 512],
                        start=(dc == 0), stop=(dc == DC - 1),
                    )
                vt = vstage.tile([P, 512], BF16, tag="vst")
                nc.vector.tensor_copy(vt[:], ps[:])
                nc.scalar.dma_start(
                    out=vin[sb * P:(sb + 1) * P, oh * 512:(oh + 1) * 512],
                    in_=vt[:])

        nc.gpsimd.collective_compute(
            "AllGather", mybir.AluOpType.bypass, replica_groups=PAIRS,
            ins=[vin.opt()], outs=[vout.opt()])

        # ---- kT reload (both halves; rank h's shard is s-half h) ----
        kt = [ktpool.tile([P, S], BF16, tag="kt", name=f"kt{i}")
              for i in range(OC)]
        for oc in range(OC):
            eng = nc.sync if oc % 2 == 0 else nc.gpsimd
            eng.dma_start(
                out=kt[oc].rearrange("p (h s) -> p h s", h=2),
                in_=kout.rearrange("(h c p) s -> c p h s", h=2, p=P)[oc])

        # ---- phase 1c: qT projection (resident) ----
        wq_sb = [load_w(wqt, i) for i in range(4)]
        xq_sb = [load_x(xq, st, eng=nc.gpsimd) for st in range(2)]
        qt = [qtpool.tile([P, SQ], BF16, tag="qt", name=f"qt{i}")
              for i in range(OC)]
        for st in range(SQ // 512):
            for oc in range(OC):
                ps = pp.tile([P, 512], F32, tag="pp")
                for dc in range(DC):
                    nc.tensor.matmul(
                        ps[:],
                        w_lhsT(wq_sb, oc, dc),
                        xq_sb[st][:, dc * 512:(dc + 1) * 512],
                        start=(dc == 0), stop=(dc == DC - 1),
                    )
                nc.vector.tensor_copy(
                    qt[oc][:, st * 512:(st + 1) * 512], ps[:])
        p1.close()

        # ---- phase 2: attention ----
        vpool = es.enter_context(tc.tile_pool(name="vpool", bufs=16))
        maskpool = es.enter_context(tc.tile_pool(name="maskpool", bufs=1))
        ptpool = es.enter_context(tc.tile_pool(name="ptpool", bufs=16))
        linvtpool = es.enter_context(tc.tile_pool(name="linvtpool", bufs=2))
        outpool = es.enter_context(tc.tile_pool(name="outpool", bufs=2))
        ps_s = es.enter_context(tc.tile_pool(name="ps_s", bufs=2, space="PSUM"))
        ps_l = es.enter_context(tc.tile_pool(name="ps_l", bufs=2, space="PSUM"))
        ps_av = es.enter_context(tc.tile_pool(name="ps_av", bufs=4, space="PSUM"))

        # the causal diagonal-band mask is identical for both groups: load
        # once; v blocks reload from the gathered buffer split across queues
        mask_sb = maskpool.tile([P, 8 * QW], BF16, tag="mask")
        nc.sync.dma_start(out=mask_sb[:], in_=mask[:])
        v_sb = []
        for kb in range(SKB):
            t = vpool.tile([P, D], BF16, tag="vres")
            eng = nc.sync if kb % 2 == 0 else nc.gpsimd
            eng.dma_start(out=t[:], in_=vout[kb * P:(kb + 1) * P, :])
            v_sb.append(t)

        # Per-group state; group g has U = 8g+8 key-block units.
        l_ps = {}
        pts = {0: [None] * 8, 1: [None] * 16}
        score_ps = {0: [None] * 8, 1: [None] * 16}
        linv_t = {}

        def emit_scores(g, j):
            # key block j contributes only to q' >= 64*(j-8g); restrict the
            # scores matmul to exactly that range (the additive mask kills
            # the stale PSUM left of it). Units 0/1 of g0 stay full-width so
            # the first use of each ps_s slot initializes the whole bank.
            jj = j - 8 * g
            rs = 64 * jj if (j >= 8 * g and (g > 0 or j >= 2)) else 0
            ps = ps_s.tile([P, QW], F32, tag="s")
            for oc in range(OC):
                nc.tensor.matmul(
                    ps[:, rs:],
                    kt[oc][:, j * P:(j + 1) * P],
                    qt[oc][:, g * QW + rs:(g + 1) * QW],
                    start=(oc == 0), stop=(oc == OC - 1),
                )
            score_ps[g][j] = ps

        def emit_post(g, j):
            # (additive causal mask) + exp + l-accumulation for unit j
            U = 8 * g + 8
            if j >= 8 * g:
                nc.vector.tensor_add(
                    score_ps[g][j][:], score_ps[g][j][:],
                    mask_sb[:, (j - 8 * g) * QW:(j - 8 * g + 1) * QW])
            pt = ptpool.tile([P, QW], BF16, tag="pt")
            nc.scalar.activation(
                pt[:], score_ps[g][j][:],
                mybir.ActivationFunctionType.Exp, scale=SCALE)
            nc.tensor.matmul(
                l_ps[g][:], ones1[:], pt[:],
                start=(j == 0), stop=(j == U - 1),
            )
            pts[g][j] = pt

        def emit_unit_range(g, lo, hi):
            for j in range(lo, hi):
                emit_scores(g, j)
                if j > lo:
                    emit_post(g, j - 1)
            emit_post(g, hi - 1)

        def emit_linv(g):
            # PSUM -> SBUF bounce (DMA can't read PSUM), transpose l into
            # lanes (4 tiny column DMAs), then reciprocal on [128, 4]
            # (a [1, 512] reciprocal on DVE costs ~4us; this path ~1.3us)
            l_sb = linvtpool.tile([1, QW], F32, tag="lsb")
            nc.vector.tensor_copy(l_sb[:], l_ps[g][:])
            lt = linvtpool.tile([P, 8], F32, tag="linvt")
            for c in range(4):
                nc.scalar.dma_start(
                    out=lt[:, c:c + 1], in_=l_sb[0:1, c * P:(c + 1) * P])
            nc.vector.reciprocal(lt[:, 4:8], lt[:, 0:4])
            linv_t[g] = lt

        def emit_av(g):
            U = 8 * g + 8
            for qs in range(4):
                jmax = min(U, 8 * g + 2 * qs + 2)
                for ot in range(2):
                    ps = ps_av.tile([P, 512], F32, tag="av")
                    for j in range(jmax):
                        nc.tensor.matmul(
                            ps[:],
                            pts[g][j][:, qs * P:(qs + 1) * P],
                            v_sb[j][:, ot * 512:(ot + 1) * 512],
                            start=(j == 0), stop=(j == jmax - 1),
                        )
                    out_sb = outpool.tile([P, 512], F32, tag="out")
                    nc.scalar.mul(out_sb[:], ps[:],
                                  linv_t[g][:, 4 + qs:5 + qs])
                    r0 = g * 4 * P + qs * P
                    nc.sync.dma_start(
                        out=out[r0:r0 + P, ot * 512:(ot + 1) * 512],
                        in_=out_sb[:])

        l_ps[0] = ps_l.tile([1, QW], F32, tag="l", name="l0")
        l_ps[1] = ps_l.tile([1, QW], F32, tag="l", name="l1")
        emit_unit_range(0, 0, 8)      # g0 scores/exp/l
        emit_linv(0)
        emit_unit_range(1, 0, 8)      # g1 first half: no v dependency, hides
        emit_av(0)                    # the v reload under these scores
        emit_unit_range(1, 8, 16)
        emit_linv(1)
        emit_av(1)

    nc.compile()
    return nc


_PROGRAM = None


def _get_program():
    global _PROGRAM
    if _PROGRAM is None:
        _PROGRAM = _build_program()
    return _PROGRAM


# Set by kernel() after each run: BassKernelResults (exec_time_ns etc.)
last_results = None


def kernel(**inputs):
    global last_results
    _install_axon_profile_hook()

    import ml_dtypes

    x = np.asarray(inputs["x"], dtype=np.float32)
    wq = np.asarray(inputs["wq"], dtype=np.float32)
    wk = np.asarray(inputs["wk"], dtype=np.float32)
    wv = np.asarray(inputs["wv"], dtype=np.float32)

    wqt = np.ascontiguousarray(wq.T).astype(ml_dtypes.bfloat16)
    wkt = np.ascontiguousarray(wk.T).astype(ml_dtypes.bfloat16)
    wvt = np.ascontiguousarray(wv.T).astype(ml_dtypes.bfloat16)

    # own query rows per core half h: parity-h rows within each 1024-row group
    own_rows = {}
    for h in range(2):
        rows = []
        for g in range(G):
            rows.extend(range(1024 * g + h, 1024 * (g + 1), 2))
        own_rows[h] = np.array(rows, dtype=np.int64)

    # additive causal mask tiles [128, 8*512] bf16:
    # tile t (= within-group key block t) masks the group's diagonal band
    masks = {}
    kl = np.arange(P)[:, None]
    qp = np.arange(QW)[None, :]
    for h in range(2):
        m = np.zeros((P, 8 * QW), dtype=np.float32)
        for t in range(8):
            krow = P * t + kl
            qrow = 2 * qp + h
            m[:, t * QW:(t + 1) * QW] = np.where(krow <= qrow, 0.0, NEG)
        masks[h] = m.astype(ml_dtypes.bfloat16)

    in_maps = []
    for c in range(N_CORES):
        b, h = divmod(c, 2)
        xt = np.ascontiguousarray(x[b].T).astype(ml_dtypes.bfloat16)  # [D, S]
        in_maps.append({
            "xh": np.ascontiguousarray(xt[:, h * SH:(h + 1) * SH]),
            "xq": np.ascontiguousarray(xt[:, own_rows[h]]),
            "wqt": wqt, "wkt": wkt, "wvt": wvt,
            "mask": masks[h],
            "ones": np.ones((P, 1), dtype=ml_dtypes.bfloat16),
        })

    nc = _get_program()
    trace = bool(int(os.environ.get("KERNEL_TRACE", "0")))
    kwargs = {}
    if trace:
        kwargs["trace"] = True
        kwargs["trace_cores"] = list(range(N_CORES))
        tdir = os.environ.get("KERNEL_TRACE_DIR")
        if tdir:
            os.makedirs(tdir, exist_ok=True)
            kwargs["tmpdir"] = tdir
    res = run_bass_kernel_spmd(nc, in_maps, core_ids=list(range(N_CORES)),
                               **kwargs)
    last_results = res

    out = np.empty((B, S, D), dtype=np.float32)
    for c in range(N_CORES):
        b, h = divmod(c, 2)
        out[b, own_rows[h], :] = res.results[c]["out"]
    return out
